# revision 21
# baseline (speedup 1.0000x reference)
"""Batch Graph-Attention layer (GAT, eval mode) on 8 Trainium2 NeuronCores.

Math per graph b (reference):
    Wh = h @ W                         (N=1024, Fo=64)
    f1 = Wh @ a1 ; f2 = Wh @ a2        (N,)
    e[i,j]   = leakyrelu(f1[i]+f2[j], 0.01)
    att      = softmax(e, axis=j)
    out      = elu(att @ Wh)

Device algorithm (per graph), avoiding any O(N^2) transcendentals AND any
O(N^2) second elementwise pass:
    exp(lrelu(x)) == max(exp(x), exp(0.01x))            (exact, slope in (0,1))
    expe[i,j] = g1[i] * max(qb[i]*rg[j], g2[j])
      qb = exp(-0.99 f1)  (broadcast [128,N] bf16)
      rg = exp( 0.01 f2),  g2 = exp(f2)   (per-chunk per-partition scalars)
    g1[i] is constant along the softmax axis -> cancels in numer/denom:
      outT[:,i] = (sum_j [Wh|1][j,:] * m[j,i]) / (same, ones row)
      m[j,i] = max(qb[i]*rg[j], g2[j])   -- ONE fused DVE tensor_scalar/chunk
    numer.T[o,i] & rowsum[i] via PE:  lhsT = [Wh | 1] bf16 (65 cols), rhs = m
    Normalization stays in the [o,i] orientation: rz-row = 1/rowsum (ACT
    Reciprocal), broadcast over 64 partitions with a K=1 PE matmul, multiply
    on DVE, elu wide on ACT/GpSimd, store TRANSPOSED; the host swapaxes the
    [B, 64, N] result back (free — outside the measured kernel).

HAM clock-gate management (measured): transposes and fp32 matmuls do NOT
count as PE activity; a ~3.4us window with >~30% PE idle re-throttles the
clock to 1.2GHz and only bf16 matmul activity re-warms it. So junk bf16
matmuls are placed at dependency wait points in the PE stream -- they
execute while the next real op waits on its semaphore, keeping the activity
monitor busy for free.

Layouts are p-major ((p c) row order) so h loads are 4KB contiguous per
partition; outT stores are 2KB contiguous. f1-broadcast matmul runs in
float32r (single-pass).

Sharding: batch dim 16 -> 8 cores x 2 graphs (pure data parallel, no comms).
"""

import numpy as np

import concourse.bass as bass
import concourse.mybir as mybir
import concourse.tile as tile
from concourse import bacc

F32 = mybir.dt.float32
F32R = mybir.dt.float32r
BF16 = mybir.dt.bfloat16
AF = mybir.ActivationFunctionType
OP = mybir.AluOpType

B_PER_CORE = 2
N = 1024
F_IN = 128
F_OUT = 64
C = N // 128  # 8 chunks of 128 rows
NEG_SLOPE = 0.01
N_WARM = 7

LAST_PERF = {}


def build_bass():
    nc = bacc.Bacc("TRN2", target_bir_lowering=False, debug=False)

    h_d = nc.dram_tensor("h", [B_PER_CORE, N, F_IN], F32, kind="ExternalInput")
    w_d = nc.dram_tensor("W", [B_PER_CORE, F_IN, F_OUT], F32, kind="ExternalInput")
    a_d = nc.dram_tensor("a", [B_PER_CORE, 1, 2 * F_OUT, 1], F32, kind="ExternalInput")
    i_d = nc.dram_tensor("ident", [128, 128], F32, kind="ExternalInput")
    o_d = nc.dram_tensor("out", [B_PER_CORE, N, F_OUT], F32, kind="ExternalOutput")

    with tile.TileContext(nc) as tc:
        with (
            tc.tile_pool(name="singles", bufs=1) as singles,
            tc.tile_pool(name="hin", bufs=2) as hin_pool,
            tc.tile_pool(name="ht", bufs=2) as ht_pool,
            tc.tile_pool(name="small", bufs=2) as small_pool,
            tc.tile_pool(name="qb", bufs=2) as qb_pool,
            tc.tile_pool(name="v", bufs=18) as v_pool,
            tc.tile_pool(name="tail", bufs=8) as tail_pool,
            tc.tile_pool(name="ps", bufs=5, space="PSUM") as ps,
            tc.tile_pool(name="pswide", bufs=1, space="PSUM") as pswide,
            tc.tile_pool(name="pswarm", bufs=1, space="PSUM") as pswarm,
        ):
            ident = singles.tile([128, 128], F32)
            nc.scalar.dma_start(out=ident[:], in_=i_d[:])
            zeros_bf = singles.tile([128, 512], BF16)
            nc.vector.memset(zeros_bf[:], 0.0)

            warm_ps = pswarm.tile([128, 512], F32, tag="warm")

            def junk(n=1):
                """bf16 heartbeat matmuls: execute during PE wait points,
                keeping the HAM activity monitor from re-throttling."""
                for _ in range(n):
                    nc.tensor.matmul(warm_ps[:], zeros_bf[:, 0:128], zeros_bf[:])

            st = [dict() for _ in range(B_PER_CORE)]

            # ---- input DMAs first so transfers start ASAP --------------
            for b in range(B_PER_CORE):
                dmae_h = nc.sync if b == 0 else nc.scalar
                dmae_w = nc.scalar if b == 0 else nc.sync
                h_sb = hin_pool.tile([128, C, F_IN], F32, tag="h")
                # p-major: partition p holds rows 8p..8p+7 -> 4KB contiguous
                dmae_h.dma_start(
                    out=h_sb[:], in_=h_d[b].rearrange("(p c) f -> p c f", p=128)
                )
                wext = small_pool.tile([128, F_OUT + 1], F32, tag="wext")
                dmae_w.dma_start(out=wext[:, 0:F_OUT], in_=w_d[b])
                apair = small_pool.tile([F_OUT, 2], F32, tag="apair")
                dmae_w.dma_start(
                    out=apair[:],
                    in_=a_d[b, 0, :, 0].rearrange("(two o) -> o two", two=2),
                )
                st[b].update(h=h_sb, wext=wext, apair=apair)

            # PE warm-up: ~3us of junk bf16 matmuls during the DMA window
            junk(N_WARM)

            def stage_a_pre(b):
                """W chain + h transposes + ht copies."""
                s = st[b]
                h_sb, wext, apair = s["h"], s["wext"], s["apair"]

                wt_ps = ps.tile([F_OUT, 128], F32, tag="ps")
                nc.tensor.transpose(wt_ps[:], wext[:, 0:F_OUT], ident[:])
                wt_sb = small_pool.tile([F_OUT, 128], F32, tag="wt")
                nc.vector.tensor_copy(wt_sb[:], wt_ps[:])
                junk(1)  # covers the wt round-trip wait
                wa_ps = ps.tile([128, 2], F32, tag="ps")
                nc.tensor.matmul(wa_ps[:], wt_sb[:], apair[:])
                wa_sb = small_pool.tile([128, 2], F32, tag="wa")
                nc.vector.tensor_copy(wa_sb[:], wa_ps[:])
                # wa2 becomes column 64 of the Wh matmul rhs -> f2 per chunk
                nc.vector.tensor_copy(wext[:, F_OUT : F_OUT + 1], wa_sb[:, 1:2])
                # broadcast wa1 along free dim -> lhsT for the f1-broadcast mm
                # (fp32r producer: DVE rounds on write)
                wa1b = small_pool.tile([128, 128], F32R, tag="wa1b")
                nc.vector.tensor_scalar(
                    wa1b[:], zeros_bf[:, 0:128], wa_sb[:, 0:1], None, op0=OP.add,
                )

                # transpose h -> ht [f, n]; PSUM->SBUF copies round to fp32r
                ht_sb = ht_pool.tile([128, N], F32R, tag="ht")
                for half in range(2):
                    pt = ps.tile([128, 4, 128], F32, tag="ps")
                    for q in range(4):
                        c = half * 4 + q
                        nc.tensor.transpose(pt[:, q, :], h_sb[:, c, :], ident[:])
                    junk(1)
                    if half == 0:
                        nc.scalar.copy(ht_sb[:, 0:512], pt[:])
                    else:
                        nc.vector.tensor_copy(ht_sb[:, 512:1024], pt[:])
                s.update(ht=ht_sb, wa1b=wa1b)

            def stage_a_mid(b):
                """Wh+f2 per chunk, f1 broadcast (PE)."""
                s = st[b]
                wext, ht_sb, wa1b = s["wext"], s["ht"], s["wa1b"]
                junk(2)  # covers the ht copy wait
                pwh = []
                for half in range(2):
                    p = ps.tile([128, 4, F_OUT + 1], F32, tag="ps")
                    pwh.append(p)
                    for q in range(4):
                        c = half * 4 + q
                        nc.tensor.matmul(
                            p[:, q, :],
                            ht_sb[:, c * 128 : (c + 1) * 128].bitcast(F32),
                            wext[:],
                        )
                    junk(1)
                # f1 broadcast: single 2-bank PSUM tile so qb is ONE wide exp
                pf1b = pswide.tile([128, N], F32, tag="pf1b")
                for half in range(2):
                    nc.tensor.matmul(
                        pf1b[:, half * 512 : (half + 1) * 512],
                        wa1b[:],
                        ht_sb[:, half * 512 : (half + 1) * 512],
                    )
                junk(1)
                s.update(pwh=pwh, pf1b=pf1b)

            def stage_a_exp1(b):
                """ACT: rg/g2 exps + qb broadcast exp (the m-op gates)."""
                s = st[b]
                pwh, pf1b = s["pwh"], s["pf1b"]
                rg = small_pool.tile([128, C], F32, tag="rg")
                g2 = small_pool.tile([128, C], F32, tag="g2")
                for half in range(2):
                    sl = slice(half * 4, (half + 1) * 4)
                    nc.scalar.activation(
                        rg[:, sl], pwh[half][:, :, F_OUT], AF.Exp, scale=NEG_SLOPE
                    )
                    nc.scalar.activation(
                        g2[:, sl], pwh[half][:, :, F_OUT], AF.Exp,
                        scale=(1.0 - NEG_SLOPE),
                    )
                # qb = exp(-0.99 f1) broadcast, one wide op
                qb = qb_pool.tile([128, N], BF16, tag="qb")
                nc.scalar.activation(
                    qb[:], pf1b[:], AF.Exp, scale=-(1.0 - NEG_SLOPE)
                )
                s.update(rg=rg, g2=g2, qb=qb)

            def stage_a_exp2(b):
                """ACT: whb = [Wh | 1] bf16 panels (needed later, by bp)."""
                s = st[b]
                pwh = s["pwh"]
                whb = small_pool.tile([128, C, F_OUT + 2], BF16, tag="whb")
                for half in range(2):
                    nc.scalar.activation(
                        whb[:, half * 4 : (half + 1) * 4, 0:F_OUT],
                        pwh[half][:, :, 0:F_OUT],
                        AF.Copy,
                    )
                nc.gpsimd.memset(whb[:, :, F_OUT : F_OUT + 2], 1.0)
                s.update(whb=whb)

            def stage_bv(b):
                """m[j,i] = max(qb[i]*rg[j], g2[j]) — one fused DVE op/chunk."""
                s = st[b]
                v_tiles = []
                for c in range(C):
                    v = v_pool.tile([128, N], BF16, tag="v")
                    v_tiles.append(v)
                    # m = max(qb, t2) * rg  ==  max(qb*rg, g2);  t2=exp(.99 f2)
                    nc.vector.tensor_scalar(
                        v[:], s["qb"][:], s["g2"][:, c : c + 1],
                        s["rg"][:, c : c + 1], op0=OP.max, op1=OP.mult,
                    )
                s["v"] = v_tiles

            def stage_bp_half(b, half):
                """8 accumulating numer.T matmuls for one 512-col half."""
                s = st[b]
                p = ps.tile([F_OUT + 1, 512], F32, tag="ps")
                s.setdefault("phpT", {})[half] = p
                for c in range(C):
                    nc.tensor.matmul(
                        p[:],
                        s["whb"][:, c, 0 : F_OUT + 1],
                        s["v"][c][:, half * 512 : (half + 1) * 512],
                        start=(c == 0),
                        stop=(c == C - 1),
                    )

            def stage_c_copy(b, half):
                """PSUM numer.T -> SBUF (ACT; overlaps next PE half)."""
                s = st[b]
                hpT = tail_pool.tile([F_OUT + 1, 512], F32, tag="hpT")
                s.setdefault("hpT", {})[half] = hpT
                nc.scalar.copy(hpT[:], s["phpT"][half][:])

            def stage_c_trans(b, half):
                """Transpose numer.T back (PE)."""
                s = st[b]
                php = ps.tile([128, 4, F_OUT + 1], F32, tag="ps")
                s.setdefault("php", {})[half] = php
                for q in range(4):
                    nc.tensor.transpose(
                        php[:, q, :],
                        s["hpT"][half][:, q * 128 : (q + 1) * 128],
                        ident[: F_OUT + 1, : F_OUT + 1],
                    )

            def stage_c_tail(b, half):
                """rz (small-shape recip), hp = php*rz (DVE), elu wide
                (ACT exp/relu + GpSimd subtract), store."""
                s = st[b]
                php = s["php"][half]
                rz = tail_pool.tile([128, 4], F32, tag="rz")
                nc.vector.reciprocal(rz[:], php[:, :, F_OUT])
                hp = tail_pool.tile([128, 4, F_OUT], F32, tag="hp")
                for q in range(4):
                    nc.vector.tensor_scalar(
                        hp[:, q, :], php[:, q, 0:F_OUT], rz[:, q : q + 1],
                        None, op0=OP.mult,
                    )
                te = tail_pool.tile([128, 4, F_OUT], F32, tag="te")
                nc.scalar.activation(te[:], hp[:], AF.Exp)
                mx = tail_pool.tile([128, 4, F_OUT], F32, tag="mx")
                nc.scalar.activation(mx[:], hp[:], AF.Relu)
                rt = tail_pool.tile([128, 4, F_OUT], F32, tag="rt")
                nc.scalar.activation(rt[:], te[:], AF.Relu, scale=-1.0, bias=1.0)
                # elu(x) = max(x,0) - relu(1 - exp(x))
                osb = tail_pool.tile([128, 4, F_OUT], F32, tag="osb")
                nc.gpsimd.tensor_tensor(osb[:], mx[:], rt[:], op=OP.subtract)
                dmae = nc.sync if half == 0 else nc.gpsimd
                dmae.dma_start(
                    out=o_d[b].rearrange("(p c) o -> p c o", p=128)[
                        :, half * 4 : (half + 1) * 4, :
                    ],
                    in_=osb[:],
                )

            # ---- emission schedule (per-engine queues fill in this order)
            stage_a_pre(0)
            stage_a_mid(0)
            stage_a_exp1(0)
            stage_a_pre(1)
            stage_a_exp2(0)
            stage_a_mid(1)
            stage_a_exp1(1)
            stage_a_exp2(1)
            stage_bv(0)
            stage_bv(1)
            junk(2)  # cover the m(0,c0) wait
            stage_bp_half(0, 0)
            stage_c_copy(0, 0)
            stage_bp_half(0, 1)
            stage_c_trans(0, 0)
            stage_c_copy(0, 1)
            stage_c_tail(0, 0)
            stage_bp_half(1, 0)
            stage_c_trans(0, 1)
            stage_c_copy(1, 0)
            stage_c_tail(0, 1)
            stage_bp_half(1, 1)
            stage_c_trans(1, 0)
            stage_c_copy(1, 1)
            stage_c_tail(1, 0)
            stage_c_trans(1, 1)
            stage_c_tail(1, 1)

    nc.compile()
    return nc


def kernel(h: np.ndarray, W: np.ndarray, a: np.ndarray, _trace: bool = False):
    from concourse.bass_utils import run_bass_kernel_spmd

    n_cores = 8
    nc = build_bass()
    ident = np.eye(128, dtype=np.float32)
    in_maps = []
    for i in range(n_cores):
        sl = slice(i * B_PER_CORE, (i + 1) * B_PER_CORE)
        in_maps.append(
            {
                "h": np.ascontiguousarray(h[sl]),
                "W": np.ascontiguousarray(W[sl]),
                "a": np.ascontiguousarray(a[sl]),
                "ident": ident,
            }
        )
    res = run_bass_kernel_spmd(
        nc, in_maps, core_ids=list(range(n_cores)), trace=_trace
    )
    LAST_PERF.clear()
    LAST_PERF.update(
        {
            "exec_time_ns": res.exec_time_ns,
            "mean_exec_time_ns": res.mean_exec_time_ns,
            "trace": res.instructions_and_trace[1]
            if res.instructions_and_trace
            else None,
        }
    )
    return np.concatenate([r["out"] for r in res.results], axis=0)


# revision 22
# speedup vs baseline: 1.2055x; 1.2055x over previous
"""Batch Graph-Attention layer (GAT, eval mode) on 8 Trainium2 NeuronCores.

Math per graph b (reference):
    Wh = h @ W                         (N=1024, Fo=64)
    f1 = Wh @ a1 ; f2 = Wh @ a2        (N,)
    e[i,j]   = leakyrelu(f1[i]+f2[j], 0.01)
    att      = softmax(e, axis=j)
    out      = elu(att @ Wh)

Device algorithm (per graph), avoiding any O(N^2) transcendentals AND any
O(N^2) second elementwise pass:
    exp(lrelu(x)) == max(exp(x), exp(0.01x))            (exact, slope in (0,1))
    expe[i,j] = g1[i] * max(qb[i]*rg[j], g2[j])
      qb = exp(-0.99 f1)  (broadcast [128,N] bf16)
      rg = exp( 0.01 f2),  g2 = exp(f2)   (per-chunk per-partition scalars)
    g1[i] is constant along the softmax axis -> cancels in numer/denom:
      outT[:,i] = (sum_j [Wh|1][j,:] * m[j,i]) / (same, ones row)
      m[j,i] = max(qb[i]*rg[j], g2[j])   -- ONE fused DVE tensor_scalar/chunk
    numer.T[o,i] & rowsum[i] via PE:  lhsT = [Wh | 1] bf16 (65 cols), rhs = m
    Normalization stays in the [o,i] orientation: rz-row = 1/rowsum (ACT
    Reciprocal), broadcast over 64 partitions with a K=1 PE matmul, multiply
    on DVE, elu wide on ACT/GpSimd, store TRANSPOSED; the host swapaxes the
    [B, 64, N] result back (free — outside the measured kernel).

HAM clock-gate management (measured): transposes and fp32 matmuls do NOT
count as PE activity; a ~3.4us window with >~30% PE idle re-throttles the
clock to 1.2GHz and only bf16 matmul activity re-warms it. So junk bf16
matmuls are placed at dependency wait points in the PE stream -- they
execute while the next real op waits on its semaphore, keeping the activity
monitor busy for free.

Layouts are p-major ((p c) row order) so h loads are 4KB contiguous per
partition; outT stores are 2KB contiguous. f1-broadcast matmul runs in
float32r (single-pass).

Sharding: batch dim 16 -> 8 cores x 2 graphs (pure data parallel, no comms).
"""

import numpy as np

import concourse.bass as bass
import concourse.mybir as mybir
import concourse.tile as tile
from concourse import bacc

F32 = mybir.dt.float32
F32R = mybir.dt.float32r
BF16 = mybir.dt.bfloat16
AF = mybir.ActivationFunctionType
OP = mybir.AluOpType

B_PER_CORE = 2
N = 1024
F_IN = 128
F_OUT = 64
C = N // 128  # 8 chunks of 128 rows
NEG_SLOPE = 0.01
N_WARM = 7

LAST_PERF = {}


def build_bass():
    nc = bacc.Bacc("TRN2", target_bir_lowering=False, debug=False)

    h_d = nc.dram_tensor("h", [B_PER_CORE, N, F_IN], F32, kind="ExternalInput")
    w_d = nc.dram_tensor("W", [B_PER_CORE, F_IN, F_OUT], F32, kind="ExternalInput")
    a_d = nc.dram_tensor("a", [B_PER_CORE, 1, 2 * F_OUT, 1], F32, kind="ExternalInput")
    i_d = nc.dram_tensor("ident", [128, 128], F32, kind="ExternalInput")
    o_d = nc.dram_tensor("out", [B_PER_CORE, N, F_OUT], F32, kind="ExternalOutput")

    with tile.TileContext(nc) as tc:
        with (
            tc.tile_pool(name="singles", bufs=1) as singles,
            tc.tile_pool(name="hin", bufs=2) as hin_pool,
            tc.tile_pool(name="ht", bufs=2) as ht_pool,
            tc.tile_pool(name="small", bufs=2) as small_pool,
            tc.tile_pool(name="qb", bufs=2) as qb_pool,
            tc.tile_pool(name="v", bufs=18) as v_pool,
            tc.tile_pool(name="tail", bufs=8) as tail_pool,
            tc.tile_pool(name="ps", bufs=5, space="PSUM") as ps,
            tc.tile_pool(name="pswide", bufs=1, space="PSUM") as pswide,
            tc.tile_pool(name="pswarm", bufs=1, space="PSUM") as pswarm,
        ):
            ident = singles.tile([128, 128], F32)
            nc.scalar.dma_start(out=ident[:], in_=i_d[:])
            zeros_bf = singles.tile([128, 512], BF16)
            nc.vector.memset(zeros_bf[:], 0.0)

            warm_ps = pswarm.tile([128, 512], F32, tag="warm")

            def junk(n=1):
                """bf16 heartbeat matmuls: execute during PE wait points,
                keeping the HAM activity monitor from re-throttling."""
                for _ in range(n):
                    nc.tensor.matmul(warm_ps[:], zeros_bf[:, 0:128], zeros_bf[:])

            st = [dict() for _ in range(B_PER_CORE)]

            # ---- input DMAs first so transfers start ASAP --------------
            for b in range(B_PER_CORE):
                dmae_h = nc.sync if b == 0 else nc.scalar
                dmae_w = nc.scalar if b == 0 else nc.sync
                h_sb = hin_pool.tile([128, C, F_IN], F32, tag="h")
                # p-major: partition p holds rows 8p..8p+7 -> 4KB contiguous
                dmae_h.dma_start(
                    out=h_sb[:], in_=h_d[b].rearrange("(p c) f -> p c f", p=128)
                )
                wext = small_pool.tile([128, F_OUT + 1], F32, tag="wext")
                dmae_w.dma_start(out=wext[:, 0:F_OUT], in_=w_d[b])
                apair = small_pool.tile([F_OUT, 2], F32, tag="apair")
                dmae_w.dma_start(
                    out=apair[:],
                    in_=a_d[b, 0, :, 0].rearrange("(two o) -> o two", two=2),
                )
                st[b].update(h=h_sb, wext=wext, apair=apair)

            # PE warm-up: ~3us of junk bf16 matmuls during the DMA window
            junk(N_WARM)

            def stage_a_pre(b):
                """W chain + h transposes + ht copies."""
                s = st[b]
                h_sb, wext, apair = s["h"], s["wext"], s["apair"]

                wt_ps = ps.tile([F_OUT, 128], F32, tag="ps")
                nc.tensor.transpose(wt_ps[:], wext[:, 0:F_OUT], ident[:])
                wt_sb = small_pool.tile([F_OUT, 128], F32, tag="wt")
                nc.vector.tensor_copy(wt_sb[:], wt_ps[:])
                junk(1)  # covers the wt round-trip wait
                wa_ps = ps.tile([128, 2], F32, tag="ps")
                nc.tensor.matmul(wa_ps[:], wt_sb[:], apair[:])
                wa_sb = small_pool.tile([128, 2], F32, tag="wa")
                nc.vector.tensor_copy(wa_sb[:], wa_ps[:])
                # wa2 becomes column 64 of the Wh matmul rhs -> f2 per chunk
                nc.vector.tensor_copy(wext[:, F_OUT : F_OUT + 1], wa_sb[:, 1:2])
                # broadcast wa1 along free dim -> lhsT for the f1-broadcast mm
                # (fp32r producer: DVE rounds on write)
                wa1b = small_pool.tile([128, 128], F32R, tag="wa1b")
                nc.vector.tensor_scalar(
                    wa1b[:], zeros_bf[:, 0:128], wa_sb[:, 0:1], None, op0=OP.add,
                )

                # transpose h -> ht [f, n]; PSUM->SBUF copies round to fp32r
                ht_sb = ht_pool.tile([128, N], F32R, tag="ht")
                for half in range(2):
                    pt = ps.tile([128, 4, 128], F32, tag="ps")
                    for q in range(4):
                        c = half * 4 + q
                        nc.tensor.transpose(pt[:, q, :], h_sb[:, c, :], ident[:])
                    junk(1)
                    if half == 0:
                        nc.scalar.copy(ht_sb[:, 0:512], pt[:])
                    else:
                        nc.vector.tensor_copy(ht_sb[:, 512:1024], pt[:])
                s.update(ht=ht_sb, wa1b=wa1b)

            def stage_a_mid(b):
                """Wh+f2 per chunk, f1 broadcast (PE)."""
                s = st[b]
                wext, ht_sb, wa1b = s["wext"], s["ht"], s["wa1b"]
                junk(2)  # covers the ht copy wait
                pwh = []
                for half in range(2):
                    p = ps.tile([128, 4, F_OUT + 1], F32, tag="ps")
                    pwh.append(p)
                    for q in range(4):
                        c = half * 4 + q
                        nc.tensor.matmul(
                            p[:, q, :],
                            ht_sb[:, c * 128 : (c + 1) * 128].bitcast(F32),
                            wext[:],
                        )
                    junk(1)
                # f1 broadcast: single 2-bank PSUM tile so qb is ONE wide exp
                pf1b = pswide.tile([128, N], F32, tag="pf1b")
                for half in range(2):
                    nc.tensor.matmul(
                        pf1b[:, half * 512 : (half + 1) * 512],
                        wa1b[:],
                        ht_sb[:, half * 512 : (half + 1) * 512],
                    )
                junk(1)
                s.update(pwh=pwh, pf1b=pf1b)

            def stage_a_exp1(b):
                """ACT: rg/g2 exps + qb broadcast exp (the m-op gates)."""
                s = st[b]
                pwh, pf1b = s["pwh"], s["pf1b"]
                rg = small_pool.tile([128, C], F32, tag="rg")
                g2 = small_pool.tile([128, C], F32, tag="g2")
                for half in range(2):
                    sl = slice(half * 4, (half + 1) * 4)
                    nc.scalar.activation(
                        rg[:, sl], pwh[half][:, :, F_OUT], AF.Exp, scale=NEG_SLOPE
                    )
                    nc.scalar.activation(
                        g2[:, sl], pwh[half][:, :, F_OUT], AF.Exp,
                        scale=(1.0 - NEG_SLOPE),
                    )
                # qb = exp(-0.99 f1) broadcast, one wide op
                qb = qb_pool.tile([128, N], BF16, tag="qb")
                nc.scalar.activation(
                    qb[:], pf1b[:], AF.Exp, scale=-(1.0 - NEG_SLOPE)
                )
                s.update(rg=rg, g2=g2, qb=qb)

            def stage_a_exp2(b):
                """ACT: whb = [Wh | 1] bf16 panels (needed later, by bp)."""
                s = st[b]
                pwh = s["pwh"]
                whb = small_pool.tile([128, C, F_OUT + 2], BF16, tag="whb")
                for half in range(2):
                    nc.scalar.activation(
                        whb[:, half * 4 : (half + 1) * 4, 0:F_OUT],
                        pwh[half][:, :, 0:F_OUT],
                        AF.Copy,
                    )
                nc.gpsimd.memset(whb[:, :, F_OUT : F_OUT + 2], 1.0)
                s.update(whb=whb)

            def stage_bv(b):
                """m[j,i] = max(qb[i]*rg[j], g2[j]) — one fused DVE op/chunk."""
                s = st[b]
                v_tiles = []
                for c in range(C):
                    v = v_pool.tile([128, N], BF16, tag="v")
                    v_tiles.append(v)
                    # m = max(qb, t2) * rg  ==  max(qb*rg, g2);  t2=exp(.99 f2)
                    nc.vector.tensor_scalar(
                        v[:], s["qb"][:], s["g2"][:, c : c + 1],
                        s["rg"][:, c : c + 1], op0=OP.max, op1=OP.mult,
                    )
                s["v"] = v_tiles

            def stage_bp_half(b, half):
                """8 accumulating numer.T matmuls for one 512-col half."""
                s = st[b]
                p = ps.tile([F_OUT + 1, 512], F32, tag="ps")
                s.setdefault("phpT", {})[half] = p
                for c in range(C):
                    nc.tensor.matmul(
                        p[:],
                        s["whb"][:, c, 0 : F_OUT + 1],
                        s["v"][c][:, half * 512 : (half + 1) * 512],
                        start=(c == 0),
                        stop=(c == C - 1),
                    )

            def stage_c_copy(b, half):
                """PSUM numer.T -> SBUF (ACT; overlaps next PE half)."""
                s = st[b]
                hpT = tail_pool.tile([F_OUT + 1, 512], F32, tag="hpT")
                s.setdefault("hpT", {})[half] = hpT
                nc.scalar.copy(hpT[:], s["phpT"][half][:])

            def stage_c_trans(b, half):
                """Transpose numer.T back (PE)."""
                s = st[b]
                php = ps.tile([128, 4, F_OUT + 1], F32, tag="ps")
                s.setdefault("php", {})[half] = php
                for q in range(4):
                    nc.tensor.transpose(
                        php[:, q, :],
                        s["hpT"][half][:, q * 128 : (q + 1) * 128],
                        ident[: F_OUT + 1, : F_OUT + 1],
                    )

            def stage_c_tail(b, half):
                """rz (small-shape recip), hp = php*rz (DVE), elu wide
                (ACT exp/relu + GpSimd subtract), store."""
                s = st[b]
                php = s["php"][half]
                rz = tail_pool.tile([128, 4, 1], F32, tag="rz")
                nc.vector.reciprocal(rz[:, :, 0], php[:, :, F_OUT])
                # hp = php * rz in ONE TT via a stride-0 broadcast view
                hp = tail_pool.tile([128, 4, F_OUT], F32, tag="hp")
                nc.vector.tensor_tensor(
                    hp[:], php[:, :, 0:F_OUT],
                    rz[:].broadcast_to([128, 4, F_OUT]), op=OP.mult,
                )
                te = tail_pool.tile([128, 4, F_OUT], F32, tag="te")
                nc.scalar.activation(te[:], hp[:], AF.Exp)
                rt = tail_pool.tile([128, 4, F_OUT], F32, tag="rt")
                nc.scalar.activation(rt[:], te[:], AF.Relu, scale=-1.0, bias=1.0)
                # elu(x) = max(x,0) - relu(1 - exp(x))
                osb = tail_pool.tile([128, 4, F_OUT], F32, tag="osb")
                nc.vector.scalar_tensor_tensor(
                    osb[:], hp[:], 0.0, rt[:], op0=OP.max, op1=OP.subtract
                )
                dmae = nc.sync if half == 0 else nc.gpsimd
                dmae.dma_start(
                    out=o_d[b].rearrange("(p c) o -> p c o", p=128)[
                        :, half * 4 : (half + 1) * 4, :
                    ],
                    in_=osb[:],
                )

            # ---- emission schedule (per-engine queues fill in this order)
            stage_a_pre(0)
            stage_a_mid(0)
            stage_a_exp1(0)
            stage_a_pre(1)
            stage_a_exp2(0)
            stage_a_mid(1)
            stage_a_exp1(1)
            stage_a_exp2(1)
            stage_bv(0)
            stage_bv(1)
            junk(2)  # cover the m(0,c0) wait
            stage_bp_half(0, 0)
            stage_c_copy(0, 0)
            stage_bp_half(0, 1)
            stage_c_trans(0, 0)
            stage_c_copy(0, 1)
            stage_c_tail(0, 0)
            stage_bp_half(1, 0)
            stage_c_trans(0, 1)
            stage_c_copy(1, 0)
            stage_c_tail(0, 1)
            stage_bp_half(1, 1)
            stage_c_trans(1, 0)
            stage_c_copy(1, 1)
            stage_c_tail(1, 0)
            stage_c_trans(1, 1)
            stage_c_tail(1, 1)

    nc.compile()
    return nc


def kernel(h: np.ndarray, W: np.ndarray, a: np.ndarray, _trace: bool = False):
    from concourse.bass_utils import run_bass_kernel_spmd

    n_cores = 8
    nc = build_bass()
    ident = np.eye(128, dtype=np.float32)
    in_maps = []
    for i in range(n_cores):
        sl = slice(i * B_PER_CORE, (i + 1) * B_PER_CORE)
        in_maps.append(
            {
                "h": np.ascontiguousarray(h[sl]),
                "W": np.ascontiguousarray(W[sl]),
                "a": np.ascontiguousarray(a[sl]),
                "ident": ident,
            }
        )
    res = run_bass_kernel_spmd(
        nc, in_maps, core_ids=list(range(n_cores)), trace=_trace
    )
    LAST_PERF.clear()
    LAST_PERF.update(
        {
            "exec_time_ns": res.exec_time_ns,
            "mean_exec_time_ns": res.mean_exec_time_ns,
            "trace": res.instructions_and_trace[1]
            if res.instructions_and_trace
            else None,
        }
    )
    return np.concatenate([r["out"] for r in res.results], axis=0)
